# revision 30
# baseline (speedup 1.0000x reference)
"""MAGNN metapath-specific layer (gather + per-edge GRU + edge-softmax +
scatter-sum) on 8 Trainium2 NeuronCores.

Strategy
--------
Host (index-only preprocessing):
  * sort edges by dst; split into 8 contiguous dst-ranges with ~E/8 edges
    each -> every core owns a disjoint output slice (no collectives).
  * pack each core's edges into 128-edge groups such that no dst segment
    crosses a group boundary; scatter row indices deduplicated to the
    first slot of each segment (PAD elsewhere, dropped by bounds check).
  * per-group segment-selection matrices S precomputed on host (bf16),
    streamed per supertile.

Device (per core, identical program, different index data):
  * indirect-DMA gather of metapath node features (3x [128,64] per group)
  * feature-major GRU in bf16: weights/h/gates bf16 (FWL weight loads,
    DVE 2x_1p elementwise); per-step xt tiles [65, 512] with a constant
    ones row so gate biases ride in the W_ih matmul (bias-free ACT ops,
    r+z sigmoid merged over a 2-bank PSUM tile); (ghn + b_hn) * r fused
    in one scalar_tensor_tensor.
  * attention logits via block-diag attn matmul; softmax without
    max-subtraction; exp via (1+tanh(a/2))/(1-tanh(a/2)); alpha
    normalization applied per-edge BEFORE aggregation (rec = 1/segsum
    gathered through the S matmul), so aggregated rows scatter directly.
  * PSUM->SBUF staging copies on gpsimd; segment sums via S matmul.
"""
import numpy as np
from contextlib import ExitStack

N_CORES = 8
GROUP = 128
PAD_ROW = 1 << 20
H, D, HD, L = 8, 64, 512, 3
LEAKY = 0.01

_RUNNER_CACHE: dict = {}


# ----------------------------------------------------------------- host plan
def _plan(edge_metapath_indices, dst, num_dst):
    E = dst.shape[0]
    order = np.argsort(dst, kind="stable")
    dst_s = dst[order].astype(np.int64)
    idx_s = edge_metapath_indices[order].astype(np.int32)

    seg_starts = np.flatnonzero(np.r_[True, dst_s[1:] != dst_s[:-1]])
    seg_ends = np.r_[seg_starts[1:], E]
    seg_sizes = seg_ends - seg_starts

    cuts = [0]
    for c in range(1, N_CORES):
        target = round(E * c / N_CORES)
        cuts.append(int(np.searchsorted(seg_starts, target, side="left")))
    cuts.append(len(seg_starts))

    cores = []
    prev_hi = 0
    for c in range(N_CORES):
        s0, s1 = cuts[c], cuts[c + 1]
        lo = prev_hi
        hi = num_dst if c == N_CORES - 1 else (
            int(dst_s[seg_starts[s1 - 1]]) + 1 if s1 > s0 else lo)
        prev_hi = hi
        groups = []
        cur, cur_edges = [], 0
        for s in range(s0, s1):
            sz = int(seg_sizes[s])
            assert sz <= GROUP, f"segment larger than a group: {sz}"
            # odd group of a pair: cap segments so the pair's total <= 128
            seg_cap = (GROUP - len(groups[-1])
                       if len(groups) % 2 == 1 else GROUP)
            if cur_edges + sz > GROUP or len(cur) >= seg_cap:
                groups.append(cur)
                cur, cur_edges = [], 0
            cur.append(s)
            cur_edges += sz
        if cur:
            groups.append(cur)
        cores.append(dict(lo=lo, hi=hi, groups=groups))

    G = max(len(ci["groups"]) for ci in cores)
    G = (G + 3) // 4 * 4
    R = max(ci["hi"] - ci["lo"] for ci in cores)

    per_core = []
    for ci in cores:
        lo = ci["lo"]
        nidx = np.zeros((128, 3 * G), np.int32)       # [edge, 3*g + t]
        drow_first = np.full((128, G), PAD_ROW, np.int32)
        seg_id = np.full((128, G), -1, np.int64)      # segment tag per slot
        seg_pos = {}
        for g, segs in enumerate(ci["groups"]):
            p = 0
            seg_pos[g] = []
            for s in segs:
                a, b = int(seg_starts[s]), int(seg_ends[s])
                n = b - a
                for t in range(L):
                    nidx[p:p + n, 3 * g + t] = idx_s[a:b, t]
                drow_first[p, g] = dst_s[a] - lo
                seg_id[p:p + n, g] = s
                seg_pos[g].append((p, n, int(dst_s[a] - lo)))
                p += n
            seg_id[p:, g] = -1000 - np.arange(128 - p)  # pad slots: self-only
        # selection matrices S[e, slot] = (seg_id[e]==seg_id[slot]), bf16
        st = np.zeros((128, 128 * G), np.float32)
        for g in range(G):
            col = seg_id[:, g]
            st[:, g * 128:(g + 1) * 128] = (col[:, None] == col[None, :])
        # packed pair scatter: S2 maps each pair's segments onto
        # disjoint slots of one 128-row tile; drow2 holds packed dst rows
        G2 = G // 2
        s2 = np.zeros((128, 128 * G), np.float32)
        drow2 = np.full((128, G2), PAD_ROW, np.int32)
        for k in range(G2):
            slot = 0
            for g in (2 * k, 2 * k + 1):
                for (p, n, dr) in seg_pos.get(g, []):
                    s2[p:p + n, g * 128 + slot] = 1.0
                    drow2[slot, k] = dr
                    slot += 1
            assert slot <= 128, f"pair {k} has {slot} segments"
        per_core.append(dict(lo=lo, hi=ci["hi"], nidx=nidx,
                             drow_first=drow_first, st=st, s2=s2,
                             drow2=drow2))
    return dict(cores=per_core, G=G, R=R)


def _to_bf16(x):
    import ml_dtypes
    return np.asarray(x, np.float32).astype(ml_dtypes.bfloat16)


def _to_fp8(x):
    import ml_dtypes
    return np.asarray(x, np.float32).astype(ml_dtypes.float8_e4m3fn)


def _host_arrays(plan, features, W_ih, W_hh, b_ih, b_hh, attn):
    """Shared (replicated) weight-derived arrays, laid out for the kernel."""
    f32 = np.float32
    bsum = (b_ih + b_hh).astype(f32)
    # wih: [128, 3*HD] = W_ih.T + bias row 64 (r: bsum, z: bsum, n: b_ih),
    # rows 65-127 zero (xt rows 65-127 are zeroed; 128 rows enable FWL)
    wih = np.zeros((128, 3 * HD), f32)
    wih[:64] = W_ih.T.astype(f32)
    wih[64, 0:HD] = bsum[0:HD]
    wih[64, HD:2 * HD] = bsum[HD:2 * HD]
    wih[64, 2 * HD:] = b_ih[2 * HD:]
    # whh in fp8 DoubleRow layout: [128, pair(2) x gatechunk(12) x ko(2) x 128]
    whhT = W_hh.T.astype(f32)                                     # [512, 1536]
    whh_dr = np.zeros((128, 2, 12, 2, 128), f32)
    for P in range(2):
        for g3 in range(3):
            for c4 in range(4):
                for j in range(2):
                    k = 2 * P + j
                    whh_dr[:, P, g3 * 4 + c4, j, :] = whhT[
                        k * 128:(k + 1) * 128,
                        g3 * HD + c4 * 128:g3 * HD + (c4 + 1) * 128]
    bhn = np.zeros((128, 4), f32)
    for c4 in range(4):
        bhn[:, c4] = b_hh[1024 + c4 * 128:1024 + (c4 + 1) * 128]
    A = np.zeros((HD, H), f32)
    for h in range(H):
        A[h * D:(h + 1) * D, h] = attn[0, h]
    attnA = np.concatenate([A[k * 128:(k + 1) * 128] for k in range(4)],
                           axis=1)                                # [128, 32]
    return dict(wih=_to_bf16(wih), whh=_to_fp8(whh_dr.reshape(128, 12 * HD)),
                bhn=bhn, attnA=_to_bf16(np.ascontiguousarray(attnA)),
                features=_to_bf16(features))


# ------------------------------------------------------------ device program
def _build_program(n_nodes, G, R):
    import concourse.bass as bass
    import concourse.tile as tile
    import concourse.mybir as mybir
    from concourse import bacc
    from concourse.masks import make_identity

    f32, i32 = mybir.dt.float32, mybir.dt.int32
    bf16 = mybir.dt.bfloat16
    fp8 = mybir.dt.float8e4
    DR = mybir.MatmulPerfMode.DoubleRow
    AF = mybir.ActivationFunctionType
    OP = mybir.AluOpType
    NST = G // 4

    nc = bacc.Bacc("TRN2", target_bir_lowering=False, debug=False,
                   num_devices=N_CORES)
    feat = nc.dram_tensor("features", [n_nodes, D], bf16, kind="ExternalInput").ap()
    wih_d = nc.dram_tensor("wih", [128, 3 * HD], bf16, kind="ExternalInput").ap()
    whh_d = nc.dram_tensor("whh", [128, 12 * HD], fp8, kind="ExternalInput").ap()
    bhn_d = nc.dram_tensor("bhn", [128, 4], f32, kind="ExternalInput").ap()
    attnA_d = nc.dram_tensor("attnA", [128, 32], bf16, kind="ExternalInput").ap()
    idx_d = nc.dram_tensor("nidx", [128, 3 * G], i32, kind="ExternalInput").ap()
    drow_d = nc.dram_tensor("drow_first", [128, G], i32, kind="ExternalInput").ap()
    st_d = nc.dram_tensor("st", [128, 128 * G], bf16, kind="ExternalInput").ap()
    s2_d = nc.dram_tensor("s2", [128, 128 * G], bf16, kind="ExternalInput").ap()
    drow2_d = nc.dram_tensor("drow2", [128, G // 2], i32,
                             kind="ExternalInput").ap()
    out_d = nc.dram_tensor("out", [R, HD], bf16, kind="ExternalOutput").ap()

    with tile.TileContext(nc) as tc, ExitStack() as ctx:
        const = ctx.enter_context(tc.tile_pool(name="const", bufs=1))
        p_mm = ctx.enter_context(tc.tile_pool(name="p_mm", bufs=3, space="PSUM"))
        p_aux = ctx.enter_context(tc.tile_pool(name="p_aux", bufs=2, space="PSUM"))
        xg_pool = ctx.enter_context(tc.tile_pool(name="xg", bufs=16))
        st_pool = ctx.enter_context(tc.tile_pool(name="stp", bufs=5))
        h_pool = ctx.enter_context(tc.tile_pool(name="h", bufs=9))
        gate_pool = ctx.enter_context(tc.tile_pool(name="gate", bufs=7))
        sm_pool = ctx.enter_context(tc.tile_pool(name="sm", bufs=4))
        out_pool = ctx.enter_context(tc.tile_pool(name="outp", bufs=4))

        ident = const.tile([128, 128], f32)
        make_identity(nc, ident[:])
        ident_bf = const.tile([128, 128], bf16)
        nc.vector.tensor_copy(ident_bf[:], ident[:])
        wih = const.tile([128, 3 * HD], bf16)
        nc.sync.dma_start(out=wih[:], in_=wih_d[:])
        whh = const.tile([128, 12 * HD], fp8)
        nc.sync.dma_start(out=whh[:], in_=whh_d[:])
        bhn = const.tile([128, 4], f32)
        nc.sync.dma_start(out=bhn[:], in_=bhn_d[:])
        attnA = const.tile([128, 32], bf16)
        nc.sync.dma_start(out=attnA[:], in_=attnA_d[:])
        idx_sb = const.tile([128, 3 * G], i32)
        nc.sync.dma_start(out=idx_sb[:], in_=idx_d[:])
        drow_sb = const.tile([128, G], i32)
        nc.sync.dma_start(out=drow_sb[:], in_=drow_d[:])
        drow2_sb = const.tile([128, G // 2], i32)
        nc.sync.dma_start(out=drow2_sb[:], in_=drow2_d[:])

        # persistent xt ring: [pair-parity][step] with constant ones row
        xt_ring = []
        for par in range(4):
            row = []
            for s in range(3):
                t = const.tile([128, 512], bf16, name=f"xt{par}{s}")
                nc.vector.memset(t[64:128, :], 0.0)
                nc.vector.memset(t[64:65, :], 1.0)
                row.append(t)
            xt_ring.append(row)

        bc = nc.gpsimd.to_reg(R - 1)

        def wih_blk(gate, c4):
            off = gate * HD + c4 * 128
            return wih[:, off:off + 128]

        def whh_dr_blk(pairP, gate, c4):
            off = (pairP * 12 + gate * 4 + c4) * 256
            return whh[:, off:off + 256].rearrange("p (ko m) -> p ko m", ko=2)

        def hview(t):                        # [128, 512] -> [128, 8, 64]
            return t.rearrange("p (h d) -> p h d", h=H)

        def emit_gather(st_i):
            """Indirect gathers + streamed S tile for one supertile."""
            g0 = 4 * st_i
            xgs = []
            for g in range(4):
                xg = xg_pool.tile([128, 3 * D], bf16, tag="xg")
                for t in range(L):
                    c = 3 * (g0 + g) + t
                    nc.gpsimd.indirect_dma_start(
                        out=xg[:, t * D:(t + 1) * D], out_offset=None,
                        in_=feat[:],
                        in_offset=bass.IndirectOffsetOnAxis(
                            ap=idx_sb[:, c:c + 1], axis=0))
                xgs.append(xg)
            st_sb = st_pool.tile([128, 512], bf16, tag="st")
            nc.sync.dma_start(out=st_sb[:],
                              in_=st_d[:, 512 * st_i:512 * (st_i + 1)])
            s2_sb = st_pool.tile([128, 512], bf16, tag="s2")
            nc.sync.dma_start(out=s2_sb[:],
                              in_=s2_d[:, 512 * st_i:512 * (st_i + 1)])
            return dict(st=st_i, xgs=xgs, st_sb=st_sb, s2_sb=s2_sb,
                        h_prev=None, h_new=None)

        def emit_xt(cx):
            """Transpose gathered features into per-step [65,512] bf16 tiles."""
            xt = xt_ring[cx["st"] % 4]
            for s in range(3):
                ps_xt = p_aux.tile([64, 512], bf16, tag="aux", name="psxt")
                for g in range(4):
                    nc.tensor.transpose(
                        out=ps_xt[:, g * 128:(g + 1) * 128],
                        in_=cx["xgs"][g][:, s * D:(s + 1) * D],
                        identity=ident_bf[:])
                nc.vector.tensor_copy(xt[s][0:64, :], ps_xt[:])
            cx["xt"] = xt

        def emit_pair1(cx, pairP):
            """GRU step 1, two chunks pair-merged (h0 = 0)."""
            xt1 = cx["xt"][0]
            omz_pair = gate_pool.tile([128, 1024], bf16, tag="zp")
            t2_pair = gate_pool.tile([128, 1024], bf16, tag="t2p")
            for c in range(2):
                c4 = 2 * pairP + c
                ps_rz = p_mm.tile([128, 1024], f32, tag="mm", name="psrz")
                nc.tensor.matmul(ps_rz[:, 0:512], lhsT=wih_blk(0, c4),
                                 rhs=xt1[:], start=True, stop=True)
                nc.tensor.matmul(ps_rz[:, 512:1024], lhsT=wih_blk(1, c4),
                                 rhs=xt1[:], start=True, stop=True)
                ps_g = p_mm.tile([128, 1024], f32, tag="mm", name="psg")
                nc.tensor.matmul(ps_g[:, 512:1024], lhsT=wih_blk(2, c4),
                                 rhs=xt1[:], start=True, stop=True)
                r_t = gate_pool.tile([128, 512], bf16, tag="r")
                nc.scalar.activation(r_t[:], ps_rz[:, 0:512], AF.Sigmoid)
                nc.scalar.activation(omz_pair[:, c * 512:(c + 1) * 512],
                                     ps_rz[:, 512:1024], AF.Sigmoid,
                                     scale=-1.0)
                nc.vector.scalar_tensor_tensor(
                    out=t2_pair[:, c * 512:(c + 1) * 512], in0=r_t[:],
                    scalar=bhn[:, c4:c4 + 1], in1=ps_g[:, 512:1024],
                    op0=OP.mult, op1=OP.add)
            n_pair = gate_pool.tile([128, 1024], bf16, tag="np")
            nc.scalar.activation(n_pair[:], t2_pair[:], AF.Tanh)
            nc.vector.tensor_mul(
                cx["h_new"][:, pairP * 1024:(pairP + 1) * 1024],
                omz_pair[:], n_pair[:])
            nc.vector.tensor_copy(
                cx["h_fp8_new"][:, pairP * 1024:(pairP + 1) * 1024],
                cx["h_new"][:, pairP * 1024:(pairP + 1) * 1024])

        def emit_pair23(cx, s, pairP):
            """GRU step 2/3, two chunks pair-merged."""
            xt_s = cx["xt"][s]
            h_prev = cx["h_prev"]
            hp8 = cx["h_fp8"]

            def hp8_pair(P):
                return hp8[:, P * 1024:(P + 1) * 1024].rearrange(
                    "p (ko n) -> p ko n", ko=2)

            z_pair = gate_pool.tile([128, 1024], bf16, tag="zp23")
            n_pair = gate_pool.tile([128, 1024], bf16, tag="np")
            ps_gs = []
            t1s = []
            for c in range(2):
                c4 = 2 * pairP + c
                ps_rz = p_mm.tile([128, 1024], f32, tag="mm", name="psrz")
                nc.tensor.matmul(ps_rz[:, 0:512], lhsT=wih_blk(0, c4),
                                 rhs=xt_s[:], start=True, stop=False)
                for P in range(2):
                    nc.tensor.matmul(
                        ps_rz[:, 0:512], lhsT=whh_dr_blk(P, 0, c4),
                        rhs=hp8_pair(P), start=False, stop=(P == 1),
                        perf_mode=DR)
                nc.tensor.matmul(ps_rz[:, 512:1024], lhsT=wih_blk(1, c4),
                                 rhs=xt_s[:], start=True, stop=False)
                for P in range(2):
                    nc.tensor.matmul(
                        ps_rz[:, 512:1024], lhsT=whh_dr_blk(P, 1, c4),
                        rhs=hp8_pair(P), start=False, stop=(P == 1),
                        perf_mode=DR)
                r_t = gate_pool.tile([128, 512], bf16, tag="r")
                nc.scalar.activation(r_t[:], ps_rz[:, 0:512], AF.Sigmoid)
                nc.scalar.activation(z_pair[:, c * 512:(c + 1) * 512],
                                     ps_rz[:, 512:1024], AF.Sigmoid)
                ps_g = p_mm.tile([128, 1024], f32, tag="mm", name="psg")
                nc.tensor.matmul(ps_g[:, 512:1024], lhsT=wih_blk(2, c4),
                                 rhs=xt_s[:], start=True, stop=False)
                for P in range(2):
                    nc.tensor.matmul(
                        ps_g[:, 0:512], lhsT=whh_dr_blk(P, 2, c4),
                        rhs=hp8_pair(P), start=(P == 0), stop=(P == 1),
                        perf_mode=DR)
                t1 = gate_pool.tile([128, 512], bf16, tag="t1")
                nc.vector.scalar_tensor_tensor(
                    out=t1[:], in0=ps_g[:, 0:512], scalar=bhn[:, c4:c4 + 1],
                    in1=r_t[:], op0=OP.add, op1=OP.mult)
                ps_gs.append(ps_g)
                t1s.append(t1)
            # t2 = gin + t1 via PE identity matmuls, hoisted after both
            # chunks' matmul blocks: the in-order PE stream must not stall
            # on chunk 0's stt latency before chunk 1's matmuls can issue
            for c in range(2):
                nc.tensor.matmul(ps_gs[c][:, 512:1024], lhsT=ident_bf[:],
                                 rhs=t1s[c][:], start=False, stop=True)
            # tanhs after both sigmoids for the same in-order reason
            for c in range(2):
                nc.scalar.activation(n_pair[:, c * 512:(c + 1) * 512],
                                     ps_gs[c][:, 512:1024], AF.Tanh)
            d_pair = gate_pool.tile([128, 1024], bf16, tag="dp")
            nc.vector.tensor_sub(
                d_pair[:], h_prev[:, pairP * 1024:(pairP + 1) * 1024],
                n_pair[:])
            zd_pair = gate_pool.tile([128, 1024], bf16, tag="zdp")
            nc.vector.tensor_mul(zd_pair[:], z_pair[:], d_pair[:])
            nc.vector.tensor_add(
                cx["h_new"][:, pairP * 1024:(pairP + 1) * 1024],
                n_pair[:], zd_pair[:])
            if s < 2:
                nc.vector.tensor_copy(
                    cx["h_fp8_new"][:, pairP * 1024:(pairP + 1) * 1024],
                    cx["h_new"][:, pairP * 1024:(pairP + 1) * 1024])

        def emit_gru(cxs, pending, nextcxs):
            outq = []
            nx = list(nextcxs)
            if nx:
                outq.append(("x", nx[0]))
            for cx in pending:
                outq.append(("h", cx))
                outq.append(("g", cx, [0, 1]))
                outq.append(("g", cx, [2, 3]))
            if len(nx) > 1:
                outq.insert(3, ("x", nx[1]))

            def drain(n):
                for _ in range(n):
                    if not outq:
                        return
                    it = outq.pop(0)
                    if it[0] == "h":
                        emit_output_head(it[1])
                    elif it[0] == "x":
                        emit_xt(it[1])
                    else:
                        emit_output_groups(it[1], it[2])

            for s in range(3):
                for cx in cxs:
                    cx["h_new"] = h_pool.tile([128, 4 * 512], bf16, tag="h",
                                              name="hplane")
                    if s < 2:
                        cx["h_fp8_new"] = h_pool.tile([128, 4 * 512], fp8,
                                                      tag="h8", name="hfp8")
                for pairP in range(2):
                    for cx in cxs:
                        if s == 0:
                            emit_pair1(cx, pairP)
                        else:
                            emit_pair23(cx, s, pairP)
                for cx in cxs:
                    cx["h_prev"] = cx["h_new"]
                    if s < 2:
                        cx["h_fp8"] = cx["h_fp8_new"]
                drain(2)
            drain(len(outq))
            for cx in cxs:
                cx["h3"] = cx["h_prev"]

        def emit_output_head(cx):
            """Attention logits + softmax weights + segment sums."""
            st_i, h3, st_sb = cx["st"], cx["h3"], cx["st_sb"]
            ps_a = p_aux.tile([128, 32], f32, tag="aux")
            for g in range(4):
                for k in range(4):
                    nc.tensor.matmul(
                        ps_a[:, g * 8:(g + 1) * 8],
                        lhsT=h3[:, k * 512 + g * 128:k * 512 + (g + 1) * 128],
                        rhs=attnA[:, k * 8:(k + 1) * 8],
                        start=(k == 0), stop=(k == 3))
            ta = sm_pool.tile([128, 32], f32, tag="ta")
            nc.vector.tensor_scalar_mul(ta[:], ps_a[:], LEAKY)
            al = sm_pool.tile([128, 32], f32, tag="al")
            nc.vector.tensor_tensor(out=al[:], in0=ps_a[:], in1=ta[:], op=OP.max)
            th = sm_pool.tile([128, 32], f32, tag="th")
            nc.scalar.activation(th[:], al[:], AF.Tanh, scale=0.5)
            nm = sm_pool.tile([128, 32], f32, tag="nm")
            nc.vector.tensor_scalar_add(nm[:], th[:], 1.0)
            dn = sm_pool.tile([128, 32], f32, tag="dn")
            nc.vector.tensor_scalar(dn[:], th[:], -1.0, 1.0, OP.mult, OP.add)
            rd = sm_pool.tile([128, 32], f32, tag="rd")
            nc.vector.reciprocal(rd[:], dn[:])
            p_st = sm_pool.tile([128, 32], bf16, tag="p")
            nc.vector.tensor_mul(p_st[:], nm[:], rd[:])

            ps_s = p_aux.tile([128, 32], f32, tag="aux", name="pss")
            for g in range(4):
                nc.tensor.matmul(ps_s[:, g * 8:(g + 1) * 8],
                                 lhsT=st_sb[:, g * 128:(g + 1) * 128],
                                 rhs=p_st[:, g * 8:(g + 1) * 8],
                                 start=True, stop=True)
            rec = sm_pool.tile([128, 32], f32, tag="rec")
            nc.vector.reciprocal(rec[:], ps_s[:])
            alpha = sm_pool.tile([128, 32], bf16, tag="alpha")
            nc.vector.tensor_mul(alpha[:], p_st[:], rec[:])
            cx["alpha"] = alpha

        def emit_output_groups(cx, gs):
            st_i, h3 = cx["st"], cx["h3"]
            s2_sb = cx["s2_sb"]
            alpha = cx["alpha"]
            pair_k = st_i * 2 + gs[0] // 2
            # both groups' weighted features first, then the two
            # accumulating matmuls back-to-back (short open-group window)
            wgs = []
            for g in gs:
                ps_h3g = p_aux.tile([128, 512], bf16, tag="aux", name="psh3g")
                for k in range(4):
                    nc.tensor.transpose(
                        out=ps_h3g[:, k * 128:(k + 1) * 128],
                        in_=h3[:, k * 512 + g * 128:k * 512 + (g + 1) * 128],
                        identity=ident_bf[:])
                wg = out_pool.tile([128, 512], bf16, tag="wg")
                nc.vector.tensor_tensor(
                    out=hview(wg[:]), in0=hview(ps_h3g[:]),
                    in1=alpha[:, g * 8:(g + 1) * 8, None].to_broadcast(
                        [128, H, D]),
                    op=OP.mult)
                wgs.append(wg)
            ps_zo = p_aux.tile([128, 512], f32, tag="aux", name="pszo")
            for i, g in enumerate(gs):
                nc.tensor.matmul(ps_zo[:],
                                 lhsT=s2_sb[:, g * 128:(g + 1) * 128],
                                 rhs=wgs[i][:], start=(i == 0),
                                 stop=(i == len(gs) - 1))
            zo = out_pool.tile([128, 512], bf16, tag="zo")
            nc.scalar.copy(zo[:], ps_zo[:])
            nc.gpsimd.indirect_dma_start(
                out=out_d[:],
                out_offset=bass.IndirectOffsetOnAxis(
                    ap=drow2_sb[:, pair_k:pair_k + 1], axis=0),
                in_=zo[:], in_offset=None,
                bounds_check=bc, oob_is_err=False)

        # pairwise-interleaved supertiles; gathers prefetched one pair
        # ahead so scatters can't head-of-line block them; outputs woven
        # into the next pair's GRU steps as PE/ACT gap fillers
        pending = []
        gq = [emit_gather(st_i) for st_i in range(0, min(2, NST))]
        for cx in gq:
            emit_xt(cx)
        for stp in range(0, NST, 2):
            cxs = gq
            gq = [emit_gather(st_i)
                  for st_i in range(stp + 2, min(stp + 4, NST))]
            emit_gru(cxs, pending, gq)
            pending = cxs
        for cx in pending:
            emit_output_head(cx)
            emit_output_groups(cx, [0, 1])
            emit_output_groups(cx, [2, 3])

    nc.compile()
    return nc


# ------------------------------------------------------------------- driver
def _get_program(n_nodes, G, R):
    key = (n_nodes, G, R)
    if key not in _RUNNER_CACHE:
        _RUNNER_CACHE[key] = _build_program(n_nodes, G, R)
    return _RUNNER_CACHE[key]


def run_on_device(plan, shared, n_nodes, trace=False):
    from concourse.bass_utils import run_bass_kernel_spmd
    nc = _get_program(n_nodes, plan["G"], plan["R"])
    in_maps = []
    for ci in plan["cores"]:
        in_maps.append({
            "features": shared["features"], "wih": shared["wih"],
            "whh": shared["whh"], "bhn": shared["bhn"],
            "attnA": shared["attnA"], "nidx": ci["nidx"],
            "drow_first": ci["drow_first"], "st": _to_bf16(ci["st"]),
            "s2": _to_bf16(ci["s2"]), "drow2": ci["drow2"],
        })
    last_err = None
    for attempt in range(4):
        try:
            return run_bass_kernel_spmd(nc, in_maps, list(range(N_CORES)),
                                        trace=trace)
        except Exception as e:          # transient NRT_EXEC_UNIT_UNRECOVERABLE
            last_err = e
            import time as _time
            _time.sleep(15)
    raise last_err


def kernel(features, W_ih, W_hh, b_ih, b_hh, attn,
           edge_metapath_indices, dst, num_dst):
    num_dst = int(num_dst)
    plan = _plan(np.asarray(edge_metapath_indices), np.asarray(dst), num_dst)
    shared = _host_arrays(plan, np.asarray(features), np.asarray(W_ih),
                          np.asarray(W_hh), np.asarray(b_ih),
                          np.asarray(b_hh), np.asarray(attn))
    res = run_on_device(plan, shared, features.shape[0])
    out = np.zeros((num_dst, HD), np.float32)
    for c, ci in enumerate(plan["cores"]):
        lo, hi = ci["lo"], ci["hi"]
        out[lo:hi] = res.results[c]["out"][:hi - lo].astype(np.float32)
    return out.reshape(num_dst, H, D)
